# revision 18
# baseline (speedup 1.0000x reference)
"""Tensor-parallel LlamaAttention (GQA + RoPE + causal) for 8 trn2 NeuronCores.

Sharding: column-parallel q/k/v by head (4 q-heads, 1 kv-head per core),
attention computed locally per head, then ROW-parallel o_proj on the local
head slice producing a partial output y_part[BT, H]; the host sums the 8
partials (the "all-reduce" of the RowParallel structure is done at gather
time on the host, so no on-device collective is needed).

Layout strategy (token dim = free dim, everything transposed):
  xT[h, t]     provided by the host (pre-transposed, bf16)
  qT/kT[d, t]  from projection matmuls (lhsT=W block, rhs=xT block)
  v[t, d]      natural layout (lhsT=xT block, rhs=Wv block)
  S^T[k, q]    = matmul(lhsT=kT slice, rhs=qT slice)
  P^T          = exp(scale*S^T) via ACT (scores bounded ~|8|, exp safe in f32)
  O^T[d, q]   += matmul(lhsT=v tile, rhs=P^T)     (PSUM accum over k)
  L[*, q]     += matmul(lhsT=ones, rhs=P^T)       (row-sums of P)
  attnT        = O^T * reciprocal_approx_fast(L)  (SBUF, bf16)
  y_part      += matmul(lhsT=attnT slice, rhs=wo rows)  -> DRAM, host-summed

Pipelining: o_proj for chunk (b,qb) is emitted after attention for the next
chunk so the PE queue never stalls waiting for the normalize (DVE) step.
"""

import math
import sys

import numpy as np

sys.path.insert(0, "/opt/trn_rl_repo")

import ml_dtypes  # noqa: E402

from concourse import bacc, bass_isa, mybir, tile  # noqa: E402
from concourse.bass_utils import run_bass_kernel_spmd  # noqa: E402

F32 = mybir.dt.float32
BF16 = mybir.dt.bfloat16
NCORES = 8
P = 128  # partitions / head dim
QB = 512  # q-block (PSUM free dim)
KB = 128  # k-block (contraction tile)

_CACHE = {}


def build_program(B, S, H, NH, NKV):
    nc = bacc.Bacc("TRN2", num_devices=NCORES)

    BT = B * S  # total tokens (4096)
    NHC = NH // NCORES  # q heads per core (4)
    DQ = NHC * P  # per-core q width (512)
    HB = H // P  # h blocks (32)
    TB = BT // QB  # token super-blocks (8)
    QBB = S // QB  # q blocks per batch (4)
    RB = QB // KB  # diag tiles per q block (4)

    xT = nc.declare_dram_parameter("xT", [H, BT], BF16, isOutput=False)
    wq_c = nc.declare_dram_parameter("wq_c", [H, DQ], BF16, isOutput=False)
    wk_c = nc.declare_dram_parameter("wk_c", [H, P], BF16, isOutput=False)
    wv_c = nc.declare_dram_parameter("wv_c", [H, P], BF16, isOutput=False)
    wo_r = nc.declare_dram_parameter("wo_r", [DQ, H], BF16, isOutput=False)
    cos_t = nc.declare_dram_parameter("cos_t", [P, BT], F32, isOutput=False)
    sinx_t = nc.declare_dram_parameter("sinx_t", [P, BT], F32, isOutput=False)
    consts_t = nc.declare_dram_parameter("consts_t", [1, P, P], BF16, isOutput=False)
    masks_t = nc.declare_dram_parameter("masks_t", [RB, P, QB], BF16, isOutput=False)
    y_c = nc.declare_dram_parameter("y_c", [BT, H], BF16, isOutput=True)

    scale = 1.0 / math.sqrt(P)

    with tile.TileContext(nc) as tc:
        with (
            tc.tile_pool(name="const", bufs=1) as constp,
            tc.tile_pool(name="persist", bufs=1) as persist,
        ):
            # triangular causal mask for the diagonal subtile: tri[k,q]=k<=q
            tri_sb = constp.tile([P, P], BF16, tag="tri")
            nc.sync.dma_start(out=tri_sb, in_=consts_t[0])
            mask_sb = [
                constp.tile([P, QB], BF16, tag=f"mask{o}", name=f"mask{o}")
                for o in range(RB)
            ]
            for o in range(RB):
                nc.sync.dma_start(out=mask_sb[o], in_=masks_t[o])

            # persistent per-core activations (bf16)
            qT = [
                persist.tile([P, BT], BF16, tag=f"qT{i}", name=f"qT{i}")
                for i in range(NHC)
            ]
            kT = persist.tile([P, BT], BF16, tag="kT")
            vt = [
                persist.tile([P, P], BF16, tag=f"v{i}", name=f"v{i}")
                for i in range(BT // P)
            ]
            # o_proj weights (rows for this core's heads), resident whole run
            wo_sb = [
                persist.tile([P, H], BF16, tag=f"wo{h}", name=f"wo{h}")
                for h in range(NHC)
            ]

            # ---------------- phase A: q/k/v projections + rope
            with (
                tc.tile_pool(name="xin", bufs=HB + 16) as xin_p,
                tc.tile_pool(name="wqkv", bufs=1) as w_p,
                tc.tile_pool(name="tabs", bufs=2) as tab_p,
                tc.tile_pool(name="ropetmp", bufs=4) as rt_p,
                tc.tile_pool(name="psq", bufs=2, space="PSUM") as psq_p,
                tc.tile_pool(name="psk", bufs=1, space="PSUM") as psk_p,
                tc.tile_pool(name="psv", bufs=2, space="PSUM") as psv_p,
            ):
                wq_sb = [
                    w_p.tile([P, DQ], BF16, tag=f"wq{i}", name=f"wq{i}")
                    for i in range(HB)
                ]
                wk_sb = [
                    w_p.tile([P, P], BF16, tag=f"wk{i}", name=f"wk{i}")
                    for i in range(HB)
                ]
                wv_sb = [
                    w_p.tile([P, P], BF16, tag=f"wv{i}", name=f"wv{i}")
                    for i in range(HB)
                ]

                def rope(dst, ps, cos_sb, sinx_sb):
                    """dst = ps*cos + shift64(ps)*sinx (all [128,QB])"""
                    t1 = rt_p.tile([P, QB], F32, tag="ropet1")
                    t2 = rt_p.tile([P, QB], F32, tag="ropet2")
                    nc.vector.tensor_tensor(t1, ps, cos_sb, mybir.AluOpType.mult)
                    h = P // 2
                    nc.vector.tensor_tensor(
                        t2[0:h], ps[h:P], sinx_sb[0:h], mybir.AluOpType.mult
                    )
                    nc.vector.tensor_tensor(
                        t2[h:P], ps[0:h], sinx_sb[h:P], mybir.AluOpType.mult
                    )
                    nc.vector.tensor_tensor(dst, t1, t2, mybir.AluOpType.add)

                for tb in range(TB):
                    t0 = tb * QB
                    cos_sb = tab_p.tile([P, QB], F32, tag="cos")
                    sinx_sb = tab_p.tile([P, QB], F32, tag="sinx")
                    nc.sync.dma_start(out=cos_sb, in_=cos_t[:, t0 : t0 + QB])
                    nc.sync.dma_start(out=sinx_sb, in_=sinx_t[:, t0 : t0 + QB])

                    # stream xT tiles for this t-chunk; on tb 0 interleave the
                    # weight loads so the PE can start as soon as pairs arrive
                    xts = []
                    for hb in range(HB):
                        if tb == 0:
                            nc.sync.dma_start(
                                out=wq_sb[hb], in_=wq_c[hb * P : (hb + 1) * P, :]
                            )
                            nc.sync.dma_start(
                                out=wk_sb[hb], in_=wk_c[hb * P : (hb + 1) * P, :]
                            )
                            nc.sync.dma_start(
                                out=wv_sb[hb], in_=wv_c[hb * P : (hb + 1) * P, :]
                            )
                        xt = xin_p.tile([P, QB], BF16, tag="xin")
                        nc.sync.dma_start(
                            out=xt, in_=xT[hb * P : (hb + 1) * P, t0 : t0 + QB]
                        )
                        xts.append(xt)
                    if tb == 0:
                        # o_proj weights: queue after phase-A-critical loads
                        for h in range(NHC):
                            nc.sync.dma_start(
                                out=wo_sb[h], in_=wo_r[h * P : (h + 1) * P, :]
                            )

                    # q projections (per 128-wide d block) + rope
                    for dq in range(NHC):
                        q_ps = psq_p.tile([P, QB], F32, tag="qps")
                        for hb in range(HB):
                            nc.tensor.matmul(
                                q_ps,
                                wq_sb[hb][:, dq * P : (dq + 1) * P],
                                xts[hb],
                                start=(hb == 0),
                                stop=(hb == HB - 1),
                            )
                        rope(qT[dq][:, t0 : t0 + QB], q_ps, cos_sb, sinx_sb)
                    # k projection + rope
                    k_ps = psk_p.tile([P, QB], F32, tag="kps")
                    for hb in range(HB):
                        nc.tensor.matmul(
                            k_ps,
                            wk_sb[hb],
                            xts[hb],
                            start=(hb == 0),
                            stop=(hb == HB - 1),
                        )
                    rope(kT[:, t0 : t0 + QB], k_ps, cos_sb, sinx_sb)
                    # v projection (natural [t, d] layout)
                    for i in range(QB // P):
                        v_ps = psv_p.tile([P, P], F32, tag="vps")
                        for hb in range(HB):
                            nc.tensor.matmul(
                                v_ps,
                                xts[hb][:, i * P : (i + 1) * P],
                                wv_sb[hb],
                                start=(hb == 0),
                                stop=(hb == HB - 1),
                            )
                        nc.scalar.copy(vt[tb * (QB // P) + i], v_ps)

            # ---------------- phases B (attention) + C (o_proj), pipelined
            # o_proj work is emitted as micro-ops drained into the attention
            # loop so the PE queue always has collective-independent work to
            # fill exp-latency stalls.
            with (
                tc.tile_pool(name="pP", bufs=4) as p_p,
                tc.tile_pool(name="lacc", bufs=3) as lacc_p,
                tc.tile_pool(name="rt2", bufs=3) as rt2_pool,
                tc.tile_pool(name="aout", bufs=3 * NHC) as ao_p,
                tc.tile_pool(name="yout", bufs=3) as y_p,
                tc.tile_pool(name="psS", bufs=2, space="PSUM") as pss_p,
                tc.tile_pool(name="psO", bufs=2, space="PSUM") as pso_p,
                tc.tile_pool(name="psY", bufs=2, space="PSUM") as psy_p,
            ):
                microq = []  # pending o_proj micro-ops

                def drain(n):
                    for _ in range(min(n, len(microq))):
                        microq.pop(0)()

                def attention(b, qb):
                    """NHC heads of attention for q tokens [qb*QB,(qb+1)*QB)
                    of batch b -> list of attn tiles [128 d, QB q] (bf16)."""
                    attn_tiles = []
                    nkb_off = qb * RB  # full (off-diagonal) k-tiles
                    q0 = b * S + qb * QB
                    for h in range(NHC):
                        o_ps = pso_p.tile([P, QB], F32, tag="ops")
                        l_acc = lacc_p.tile([P, QB], F32, tag="lacc")
                        kb = 0
                        while kb < nkb_off:  # off-diagonal, paired exp
                            w = 2 if kb + 1 < nkb_off else 1
                            s_ps = pss_p.tile([P, 2 * QB], F32, tag="sps")
                            for u in range(w):
                                k0 = b * S + (kb + u) * KB
                                nc.tensor.matmul(
                                    s_ps[:, u * QB : u * QB + QB],
                                    kT[:, k0 : k0 + KB],
                                    qT[h][:, q0 : q0 + QB],
                                    start=True,
                                    stop=True,
                                )
                            p_sb = p_p.tile([P, 2 * QB], BF16, tag="P")
                            nc.scalar.activation(
                                p_sb[:, 0 : w * QB],
                                s_ps[:, 0 : w * QB],
                                mybir.ActivationFunctionType.Exp,
                                scale=scale,
                            )
                            for u in range(w):
                                ph = p_sb[:, u * QB : u * QB + QB]
                                nc.tensor.matmul(
                                    o_ps,
                                    vt[(b * S + (kb + u) * KB) // P],
                                    ph,
                                    start=(kb + u == 0),
                                    stop=False,
                                )
                                if kb + u == 0:
                                    nc.vector.tensor_copy(l_acc, ph)
                                else:
                                    nc.vector.tensor_tensor(
                                        l_acc, l_acc, ph, mybir.AluOpType.add
                                    )
                            kb += w
                            drain(1)
                        # diagonal supertile (full-width tiles, masked)
                        for o in range(RB):
                            kb_d = qb * RB + o
                            k0 = b * S + kb_d * KB
                            s_ps = pss_p.tile([P, 2 * QB], F32, tag="sps")
                            nc.tensor.matmul(
                                s_ps[:, 0:QB],
                                kT[:, k0 : k0 + KB],
                                qT[h][:, q0 : q0 + QB],
                                start=True,
                                stop=True,
                            )
                            p_sb = p_p.tile([P, 2 * QB], BF16, tag="P")
                            nc.scalar.activation(
                                p_sb[:, 0:QB],
                                s_ps[:, 0:QB],
                                mybir.ActivationFunctionType.Exp,
                                scale=scale,
                            )
                            nc.vector.tensor_tensor(
                                p_sb[:, 0:QB],
                                p_sb[:, 0:QB],
                                mask_sb[o],
                                mybir.AluOpType.mult,
                            )
                            if kb_d == 0:
                                nc.vector.tensor_copy(l_acc, p_sb[:, 0:QB])
                            else:
                                nc.vector.tensor_tensor(
                                    l_acc,
                                    l_acc,
                                    p_sb[:, 0:QB],
                                    mybir.AluOpType.add,
                                )
                            nc.tensor.matmul(
                                o_ps,
                                vt[k0 // P],
                                p_sb[:, 0:QB],
                                start=(kb_d == 0),
                                stop=(o == RB - 1),
                            )
                            drain(1)
                        l_red = rt2_pool.tile([P, QB], F32, tag="lred")
                        nc.gpsimd.partition_all_reduce(
                            l_red, l_acc, 128, bass_isa.ReduceOp.add
                        )
                        rinv = rt2_pool.tile([P, QB], F32, tag="rinv")
                        nc.vector.reciprocal_approx_fast(out=rinv, in_=l_red)
                        attn_sb = ao_p.tile([P, QB], BF16, tag="attn")
                        nc.vector.tensor_tensor(
                            attn_sb, o_ps, rinv, mybir.AluOpType.mult
                        )
                        attn_tiles.append(attn_sb)
                        drain(1)
                    return attn_tiles

                def push_oproj(b, qb, attn_tiles):
                    """Queue partial o_proj micro-ops for chunk (b, qb):
                    y[t0+ti*P, :] = sum_h attn_h[:, ti]^T @ wo_rows_h."""
                    t0 = b * S + qb * QB
                    for ti in range(QB // P):
                        holder = {}
                        for nch in range(H // QB):

                            def op(ti=ti, nch=nch, holder=holder, tiles=attn_tiles, t0=t0):
                                if nch == 0:
                                    holder["y"] = y_p.tile(
                                        [P, H], BF16, tag="ysb", name="ysb"
                                    )
                                y_sb = holder["y"]
                                y_ps = psy_p.tile([P, QB], F32, tag="yps")
                                for h in range(NHC):
                                    nc.tensor.matmul(
                                        y_ps,
                                        tiles[h][:, ti * P : (ti + 1) * P],
                                        wo_sb[h][:, nch * QB : (nch + 1) * QB],
                                        start=(h == 0),
                                        stop=(h == NHC - 1),
                                    )
                                dst = y_sb[:, nch * QB : (nch + 1) * QB]
                                if nch % 2 == 0:
                                    nc.scalar.copy(dst, y_ps)
                                else:
                                    nc.vector.tensor_copy(dst, y_ps)
                                if nch == H // QB - 1:
                                    nc.sync.dma_start(
                                        out=y_c[t0 + ti * P : t0 + (ti + 1) * P, :],
                                        in_=y_sb,
                                    )

                            microq.append(op)

                chunks = [(b, qb) for b in range(B) for qb in range(QBB)]
                for b, qb in chunks:
                    tiles = attention(b, qb)
                    push_oproj(b, qb, tiles)
                drain(len(microq))
    nc.finalize()
    return nc


def _prep_inputs(hidden_states, wq, wk, wv, wo, position_ids, B, S, H, NH, NKV):
    """Host-side: bf16 casts, x transpose, rope tables, masks, per-core slices."""
    BT = B * S
    NHC = NH // NCORES
    DQ = NHC * P
    RB = QB // KB

    bf = ml_dtypes.bfloat16
    xT = np.ascontiguousarray(
        np.asarray(hidden_states).reshape(BT, H).T
    ).astype(bf)
    wq_b, wk_b, wv_b, wo_b = (np.asarray(w).astype(bf) for w in (wq, wk, wv, wo))

    # rope tables in transposed layout [128 d, BT t]
    half = P // 2
    inv_freq = 1.0 / (10000.0 ** (np.arange(half, dtype=np.float64) / half))
    pos = np.asarray(position_ids).astype(np.float64).reshape(BT)
    ang = pos[None, :] * inv_freq[:, None]  # [64, BT]
    cos_t = np.concatenate([np.cos(ang), np.cos(ang)], 0).astype(np.float32)
    sinx_t = np.concatenate([-np.sin(ang), np.sin(ang)], 0).astype(np.float32)

    # triangular causal mask: tri[k, q] = 1 if k <= q
    consts_t = (
        np.arange(P)[None, :, None] <= np.arange(P)[None, None, :]
    ).astype(bf)
    # full-width diagonal-block masks: mask[o][k, q] = 1 if o*KB + k <= q
    kk = np.arange(KB)[None, :, None]
    qq = np.arange(QB)[None, None, :]
    oo = np.arange(RB)[:, None, None]
    masks_t = ((oo * KB + kk) <= qq).astype(bf)

    in_maps = []
    for c in range(NCORES):
        in_maps.append(
            {
                "xT": xT,
                "wq_c": np.ascontiguousarray(wq_b[:, c * DQ : (c + 1) * DQ]),
                "wk_c": np.ascontiguousarray(wk_b[:, c * P : (c + 1) * P]),
                "wv_c": np.ascontiguousarray(wv_b[:, c * P : (c + 1) * P]),
                "wo_r": np.ascontiguousarray(wo_b[c * DQ : (c + 1) * DQ, :]),
                "cos_t": cos_t,
                "sinx_t": sinx_t,
                "consts_t": consts_t,
                "masks_t": masks_t,
            }
        )
    return in_maps


def run(hidden_states, wq, wk, wv, wo, position_ids, B, S, H, NH, NKV, trace=False):
    key = (B, S, H, NH, NKV)
    if key not in _CACHE:
        _CACHE[key] = build_program(B, S, H, NH, NKV)
    nc = _CACHE[key]
    in_maps = _prep_inputs(
        hidden_states, wq, wk, wv, wo, position_ids, B, S, H, NH, NKV
    )
    res = run_bass_kernel_spmd(nc, in_maps, core_ids=list(range(NCORES)), trace=trace)
    acc = np.zeros((B * S, H), np.float32)
    for c in range(NCORES):
        acc += np.asarray(res.results[c]["y_c"], dtype=np.float32)
    out = acc.reshape(B, S, H)
    return (out, res) if trace else (out, None)


def kernel(hidden_states, wq, wk, wv, wo, position_ids):
    out, _ = run(
        hidden_states, wq, wk, wv, wo, position_ids, 2, 2048, 4096, 32, 8
    )
    return out


# revision 23
# speedup vs baseline: 1.0330x; 1.0330x over previous
"""Tensor-parallel LlamaAttention (GQA + RoPE + causal) for 8 trn2 NeuronCores.

Sharding: column-parallel q/k/v by head (4 q-heads, 1 kv-head per core),
attention computed locally per head, then ROW-parallel o_proj on the local
head slice producing a partial output y_part[BT, H]; the host sums the 8
partials (the "all-reduce" of the RowParallel structure is done at gather
time on the host, so no on-device collective is needed).

Layout strategy (token dim = free dim, everything transposed):
  xT[h, t]     provided by the host (pre-transposed, bf16)
  qT/kT[d, t]  from projection matmuls (lhsT=W block, rhs=xT block)
  v[t, d]      natural layout (lhsT=xT block, rhs=Wv block)
  S^T[k, q]    = matmul(lhsT=kT slice, rhs=qT slice)
  P^T          = exp(scale*S^T) via ACT (scores bounded ~|8|, exp safe in f32)
  O^T[d, q]   += matmul(lhsT=v tile, rhs=P^T)     (PSUM accum over k)
  L[*, q]     += matmul(lhsT=ones, rhs=P^T)       (row-sums of P)
  attnT        = O^T * reciprocal_approx_fast(L)  (SBUF, bf16)
  y_part      += matmul(lhsT=attnT slice, rhs=wo rows)  -> DRAM, host-summed

Pipelining: o_proj for chunk (b,qb) is emitted after attention for the next
chunk so the PE queue never stalls waiting for the normalize (DVE) step.
"""

import math
import sys

import numpy as np

sys.path.insert(0, "/opt/trn_rl_repo")

import ml_dtypes  # noqa: E402

from concourse import bacc, bass_isa, mybir, tile  # noqa: E402
from concourse.bass_utils import run_bass_kernel_spmd  # noqa: E402

F32 = mybir.dt.float32
BF16 = mybir.dt.bfloat16
NCORES = 8
P = 128  # partitions / head dim
QB = 512  # q-block (PSUM free dim)
KB = 128  # k-block (contraction tile)

_CACHE = {}


def build_program(B, S, H, NH, NKV):
    nc = bacc.Bacc("TRN2", num_devices=NCORES)

    BT = B * S  # total tokens (4096)
    NHC = NH // NCORES  # q heads per core (4)
    DQ = NHC * P  # per-core q width (512)
    HB = H // P  # h blocks (32)
    TB = BT // QB  # token super-blocks (8)
    QBB = S // QB  # q blocks per batch (4)
    RB = QB // KB  # diag tiles per q block (4)

    xT = nc.declare_dram_parameter("xT", [H, BT], BF16, isOutput=False)
    wq_c = nc.declare_dram_parameter("wq_c", [H, DQ], BF16, isOutput=False)
    wk_c = nc.declare_dram_parameter("wk_c", [H, P], BF16, isOutput=False)
    wv_c = nc.declare_dram_parameter("wv_c", [H, P], BF16, isOutput=False)
    wo_r = nc.declare_dram_parameter("wo_r", [DQ, H], BF16, isOutput=False)
    cos_t = nc.declare_dram_parameter("cos_t", [P, BT], F32, isOutput=False)
    sinx_t = nc.declare_dram_parameter("sinx_t", [P, BT], F32, isOutput=False)
    consts_t = nc.declare_dram_parameter("consts_t", [1, P, P], BF16, isOutput=False)
    masks_t = nc.declare_dram_parameter("masks_t", [RB, P, QB], BF16, isOutput=False)
    y_c = nc.declare_dram_parameter("y_c", [BT, H], BF16, isOutput=True)

    scale = 1.0 / math.sqrt(P)

    with tile.TileContext(nc) as tc:
        with (
            tc.tile_pool(name="const", bufs=1) as constp,
            tc.tile_pool(name="persist", bufs=1) as persist,
        ):
            # ones matrix for the L (softmax denominator) row-sum matmuls
            ones_sb = constp.tile([P, P], BF16, tag="ones")
            nc.sync.dma_start(out=ones_sb, in_=consts_t[0])
            mask_sb = [
                constp.tile([P, QB], BF16, tag=f"mask{o}", name=f"mask{o}")
                for o in range(RB)
            ]
            for o in range(RB):
                nc.sync.dma_start(out=mask_sb[o], in_=masks_t[o])

            # persistent per-core activations (bf16)
            qT = [
                persist.tile([P, BT], BF16, tag=f"qT{i}", name=f"qT{i}")
                for i in range(NHC)
            ]
            kT = persist.tile([P, BT], BF16, tag="kT")
            vt = [
                persist.tile([P, P], BF16, tag=f"v{i}", name=f"v{i}")
                for i in range(BT // P)
            ]
            # o_proj weights (rows for this core's heads), resident whole run
            wo_sb = [
                persist.tile([P, H], BF16, tag=f"wo{h}", name=f"wo{h}")
                for h in range(NHC)
            ]

            # ---------------- phase A: q/k/v projections + rope
            with (
                tc.tile_pool(name="xin", bufs=HB + 16) as xin_p,
                tc.tile_pool(name="wqkv", bufs=1) as w_p,
                tc.tile_pool(name="tabs", bufs=2) as tab_p,
                tc.tile_pool(name="ropetmp", bufs=4) as rt_p,
                tc.tile_pool(name="psq", bufs=2, space="PSUM") as psq_p,
                tc.tile_pool(name="psk", bufs=1, space="PSUM") as psk_p,
                tc.tile_pool(name="psv", bufs=2, space="PSUM") as psv_p,
            ):
                wq_sb = [
                    w_p.tile([P, DQ], BF16, tag=f"wq{i}", name=f"wq{i}")
                    for i in range(HB)
                ]
                wk_sb = [
                    w_p.tile([P, P], BF16, tag=f"wk{i}", name=f"wk{i}")
                    for i in range(HB)
                ]
                wv_sb = [
                    w_p.tile([P, P], BF16, tag=f"wv{i}", name=f"wv{i}")
                    for i in range(HB)
                ]

                def rope(dst, ps, cos_sb, sinx_sb):
                    """dst = ps*cos + shift64(ps)*sinx (all [128,QB])"""
                    t1 = rt_p.tile([P, QB], F32, tag="ropet1")
                    t2 = rt_p.tile([P, QB], F32, tag="ropet2")
                    nc.vector.tensor_tensor(t1, ps, cos_sb, mybir.AluOpType.mult)
                    h = P // 2
                    nc.vector.tensor_tensor(
                        t2[0:h], ps[h:P], sinx_sb[0:h], mybir.AluOpType.mult
                    )
                    nc.vector.tensor_tensor(
                        t2[h:P], ps[0:h], sinx_sb[h:P], mybir.AluOpType.mult
                    )
                    nc.vector.tensor_tensor(dst, t1, t2, mybir.AluOpType.add)

                for tb in range(TB):
                    t0 = tb * QB
                    cos_sb = tab_p.tile([P, QB], F32, tag="cos")
                    sinx_sb = tab_p.tile([P, QB], F32, tag="sinx")
                    nc.sync.dma_start(out=cos_sb, in_=cos_t[:, t0 : t0 + QB])
                    nc.sync.dma_start(out=sinx_sb, in_=sinx_t[:, t0 : t0 + QB])

                    # stream xT tiles for this t-chunk; on tb 0 interleave the
                    # weight loads so the PE can start as soon as pairs arrive
                    xts = []
                    for hb in range(HB):
                        if tb == 0:
                            nc.sync.dma_start(
                                out=wq_sb[hb], in_=wq_c[hb * P : (hb + 1) * P, :]
                            )
                            nc.sync.dma_start(
                                out=wk_sb[hb], in_=wk_c[hb * P : (hb + 1) * P, :]
                            )
                            nc.sync.dma_start(
                                out=wv_sb[hb], in_=wv_c[hb * P : (hb + 1) * P, :]
                            )
                        xt = xin_p.tile([P, QB], BF16, tag="xin")
                        nc.sync.dma_start(
                            out=xt, in_=xT[hb * P : (hb + 1) * P, t0 : t0 + QB]
                        )
                        xts.append(xt)
                    if tb == 0:
                        # o_proj weights: queue after phase-A-critical loads
                        for h in range(NHC):
                            nc.sync.dma_start(
                                out=wo_sb[h], in_=wo_r[h * P : (h + 1) * P, :]
                            )

                    # q projections (per 128-wide d block) + rope
                    for dq in range(NHC):
                        q_ps = psq_p.tile([P, QB], F32, tag="qps")
                        for hb in range(HB):
                            nc.tensor.matmul(
                                q_ps,
                                wq_sb[hb][:, dq * P : (dq + 1) * P],
                                xts[hb],
                                start=(hb == 0),
                                stop=(hb == HB - 1),
                            )
                        rope(qT[dq][:, t0 : t0 + QB], q_ps, cos_sb, sinx_sb)
                    # k projection + rope
                    k_ps = psk_p.tile([P, QB], F32, tag="kps")
                    for hb in range(HB):
                        nc.tensor.matmul(
                            k_ps,
                            wk_sb[hb],
                            xts[hb],
                            start=(hb == 0),
                            stop=(hb == HB - 1),
                        )
                    rope(kT[:, t0 : t0 + QB], k_ps, cos_sb, sinx_sb)
                    # v projection (natural [t, d] layout)
                    for i in range(QB // P):
                        v_ps = psv_p.tile([P, P], F32, tag="vps")
                        for hb in range(HB):
                            nc.tensor.matmul(
                                v_ps,
                                xts[hb][:, i * P : (i + 1) * P],
                                wv_sb[hb],
                                start=(hb == 0),
                                stop=(hb == HB - 1),
                            )
                        nc.scalar.copy(vt[tb * (QB // P) + i], v_ps)

            # ---------------- phases B (attention) + C (o_proj), pipelined
            # o_proj work is emitted as micro-ops drained into the attention
            # loop so the PE queue always has collective-independent work to
            # fill exp-latency stalls.
            with (
                tc.tile_pool(name="pP", bufs=4) as p_p,
                tc.tile_pool(name="rt2", bufs=3) as rt2_pool,
                tc.tile_pool(name="aout", bufs=3 * NHC) as ao_p,
                tc.tile_pool(name="yout", bufs=3) as y_p,
                tc.tile_pool(name="psS", bufs=2, space="PSUM") as pss_p,
                tc.tile_pool(name="psO", bufs=2, space="PSUM") as pso_p,
                tc.tile_pool(name="psL", bufs=2, space="PSUM") as psl_p,
                tc.tile_pool(name="psY", bufs=2, space="PSUM") as psy_p,
            ):
                microq = []  # pending o_proj micro-ops

                def drain(n):
                    for _ in range(min(n, len(microq))):
                        microq.pop(0)()

                def attention(b, qb):
                    """NHC heads of attention for q tokens [qb*QB,(qb+1)*QB)
                    of batch b -> list of attn tiles [128 d, QB q] (bf16)."""
                    attn_tiles = []
                    q0 = b * S + qb * QB
                    nkb = (qb + 1) * RB
                    for h in range(NHC):
                        o_ps = pso_p.tile([P, QB], F32, tag="ops")
                        l_ps = psl_p.tile([P, QB], F32, tag="lps")
                        for kb in range(nkb):
                            k0 = b * S + kb * KB
                            s_ps = pss_p.tile([P, QB], F32, tag="sps")
                            nc.tensor.matmul(
                                s_ps,
                                kT[:, k0 : k0 + KB],
                                qT[h][:, q0 : q0 + QB],
                                start=True,
                                stop=True,
                            )
                            p_sb = p_p.tile([P, QB], BF16, tag="P")
                            nc.scalar.activation(
                                p_sb,
                                s_ps,
                                mybir.ActivationFunctionType.Exp,
                                scale=scale,
                            )
                            o = kb - qb * RB
                            if o >= 0:
                                nc.vector.tensor_tensor(
                                    p_sb, p_sb, mask_sb[o], mybir.AluOpType.mult
                                )
                            nc.tensor.matmul(
                                o_ps,
                                vt[k0 // P],
                                p_sb,
                                start=(kb == 0),
                                stop=(kb == nkb - 1),
                            )
                            nc.tensor.matmul(
                                l_ps,
                                ones_sb,
                                p_sb,
                                start=(kb == 0),
                                stop=(kb == nkb - 1),
                            )
                            drain(1)
                        rinv = rt2_pool.tile([P, QB], F32, tag="rinv")
                        nc.vector.reciprocal_approx_fast(out=rinv, in_=l_ps)
                        attn_sb = ao_p.tile([P, QB], BF16, tag="attn")
                        nc.vector.tensor_tensor(
                            attn_sb, o_ps, rinv, mybir.AluOpType.mult
                        )
                        attn_tiles.append(attn_sb)
                        drain(1)
                    return attn_tiles

                def push_oproj(b, qb, attn_tiles):
                    """Queue partial o_proj micro-ops for chunk (b, qb):
                    y[t0+ti*P, :] = sum_h attn_h[:, ti]^T @ wo_rows_h."""
                    t0 = b * S + qb * QB
                    for ti in range(QB // P):
                        holder = {}
                        for nch in range(H // QB):

                            def op(ti=ti, nch=nch, holder=holder, tiles=attn_tiles, t0=t0):
                                if nch == 0:
                                    holder["y"] = y_p.tile(
                                        [P, H], BF16, tag="ysb", name="ysb"
                                    )
                                y_sb = holder["y"]
                                y_ps = psy_p.tile([P, QB], F32, tag="yps")
                                for h in range(NHC):
                                    nc.tensor.matmul(
                                        y_ps,
                                        tiles[h][:, ti * P : (ti + 1) * P],
                                        wo_sb[h][:, nch * QB : (nch + 1) * QB],
                                        start=(h == 0),
                                        stop=(h == NHC - 1),
                                    )
                                dst = y_sb[:, nch * QB : (nch + 1) * QB]
                                if nch % 2 == 0:
                                    nc.scalar.copy(dst, y_ps)
                                else:
                                    nc.vector.tensor_copy(dst, y_ps)
                                if nch == H // QB - 1:
                                    nc.sync.dma_start(
                                        out=y_c[t0 + ti * P : t0 + (ti + 1) * P, :],
                                        in_=y_sb,
                                    )

                            microq.append(op)

                chunks = [(b, qb) for b in range(B) for qb in range(QBB)]
                for b, qb in chunks:
                    tiles = attention(b, qb)
                    push_oproj(b, qb, tiles)
                drain(len(microq))
    nc.finalize()
    return nc


def _prep_inputs(hidden_states, wq, wk, wv, wo, position_ids, B, S, H, NH, NKV):
    """Host-side: bf16 casts, x transpose, rope tables, masks, per-core slices."""
    BT = B * S
    NHC = NH // NCORES
    DQ = NHC * P
    RB = QB // KB

    bf = ml_dtypes.bfloat16
    xT = np.ascontiguousarray(
        np.asarray(hidden_states).reshape(BT, H).T
    ).astype(bf)
    wq_b, wk_b, wv_b, wo_b = (np.asarray(w).astype(bf) for w in (wq, wk, wv, wo))

    # rope tables in transposed layout [128 d, BT t]
    half = P // 2
    inv_freq = 1.0 / (10000.0 ** (np.arange(half, dtype=np.float64) / half))
    pos = np.asarray(position_ids).astype(np.float64).reshape(BT)
    ang = pos[None, :] * inv_freq[:, None]  # [64, BT]
    cos_t = np.concatenate([np.cos(ang), np.cos(ang)], 0).astype(np.float32)
    sinx_t = np.concatenate([-np.sin(ang), np.sin(ang)], 0).astype(np.float32)

    consts_t = np.ones((1, P, P)).astype(bf)
    # full-width diagonal-block masks: mask[o][k, q] = 1 if o*KB + k <= q
    kk = np.arange(KB)[None, :, None]
    qq = np.arange(QB)[None, None, :]
    oo = np.arange(RB)[:, None, None]
    masks_t = ((oo * KB + kk) <= qq).astype(bf)

    in_maps = []
    for c in range(NCORES):
        in_maps.append(
            {
                "xT": xT,
                "wq_c": np.ascontiguousarray(wq_b[:, c * DQ : (c + 1) * DQ]),
                "wk_c": np.ascontiguousarray(wk_b[:, c * P : (c + 1) * P]),
                "wv_c": np.ascontiguousarray(wv_b[:, c * P : (c + 1) * P]),
                "wo_r": np.ascontiguousarray(wo_b[c * DQ : (c + 1) * DQ, :]),
                "cos_t": cos_t,
                "sinx_t": sinx_t,
                "consts_t": consts_t,
                "masks_t": masks_t,
            }
        )
    return in_maps


def run(hidden_states, wq, wk, wv, wo, position_ids, B, S, H, NH, NKV, trace=False):
    key = (B, S, H, NH, NKV)
    if key not in _CACHE:
        _CACHE[key] = build_program(B, S, H, NH, NKV)
    nc = _CACHE[key]
    in_maps = _prep_inputs(
        hidden_states, wq, wk, wv, wo, position_ids, B, S, H, NH, NKV
    )
    res = run_bass_kernel_spmd(nc, in_maps, core_ids=list(range(NCORES)), trace=trace)
    acc = np.zeros((B * S, H), np.float32)
    for c in range(NCORES):
        acc += np.asarray(res.results[c]["y_c"], dtype=np.float32)
    out = acc.reshape(B, S, H)
    return (out, res) if trace else (out, None)


def kernel(hidden_states, wq, wk, wv, wo, position_ids):
    out, _ = run(
        hidden_states, wq, wk, wv, wo, position_ids, 2, 2048, 4096, 32, 8
    )
    return out


# revision 24
# speedup vs baseline: 1.1173x; 1.0817x over previous
"""Tensor-parallel LlamaAttention (GQA + RoPE + causal) for 8 trn2 NeuronCores.

Sharding: column-parallel q/k/v by head (4 q-heads, 1 kv-head per core),
attention computed locally per head, then ROW-parallel o_proj on the local
head slice producing a partial output y_part[BT, H]; the host sums the 8
partials (the "all-reduce" of the RowParallel structure is done at gather
time on the host, so no on-device collective is needed).

Layout strategy (token dim = free dim, everything transposed):
  xT[h, t]     provided by the host (pre-transposed, bf16)
  qT/kT[d, t]  from projection matmuls (lhsT=W block, rhs=xT block)
  v[t, d]      natural layout (lhsT=xT block, rhs=Wv block)
  S^T[k, q]    = matmul(lhsT=kT slice, rhs=qT slice)
  P^T          = exp(scale*S^T) via ACT (scores bounded ~|8|, exp safe)
  O^T[d, q]   += matmul(lhsT=v tile, rhs=P^T)    (PSUM accum over k)
  L (denoms)   = two bf16 DVE accumulators of P tiles, then a tiny
                 ones-matmul on PE broadcasts the partition-sum into PSUM
  attnT        = O^T * reciprocal_approx_fast(L) (SBUF, bf16)
  y_part      += matmul(lhsT=attnT slice, rhs=wo rows) -> DRAM, host-summed

Pipelining: projection chunk i (512 tokens), attention chunk i, and o_proj
micro-ops for chunk i-1 are interleaved in the instruction stream, so the
PE always has independent work during DMA warmup and exp-latency stalls.
"""

import math
import sys

import numpy as np

sys.path.insert(0, "/opt/trn_rl_repo")

import ml_dtypes  # noqa: E402

from concourse import bacc, mybir, tile  # noqa: E402
from concourse.bass_utils import run_bass_kernel_spmd  # noqa: E402

F32 = mybir.dt.float32
BF16 = mybir.dt.bfloat16
NCORES = 8
P = 128  # partitions / head dim
QB = 512  # q-block (PSUM free dim)
KB = 128  # k-block (contraction tile)

_CACHE = {}


def build_program(B, S, H, NH, NKV):
    nc = bacc.Bacc("TRN2", num_devices=NCORES)

    BT = B * S  # total tokens (4096)
    NHC = NH // NCORES  # q heads per core (4)
    DQ = NHC * P  # per-core q width (512)
    HB = H // P  # h blocks (32)
    TB = BT // QB  # token super-blocks (8)
    QBB = S // QB  # q blocks per batch (4)
    RB = QB // KB  # diag tiles per q block (4)

    xT = nc.declare_dram_parameter("xT", [H, BT], BF16, isOutput=False)
    wq_c = nc.declare_dram_parameter("wq_c", [H, DQ], BF16, isOutput=False)
    wk_c = nc.declare_dram_parameter("wk_c", [H, P], BF16, isOutput=False)
    wv_c = nc.declare_dram_parameter("wv_c", [H, P], BF16, isOutput=False)
    wo_r = nc.declare_dram_parameter("wo_r", [DQ, H], BF16, isOutput=False)
    cos_t = nc.declare_dram_parameter("cos_t", [P, BT], F32, isOutput=False)
    sinx_t = nc.declare_dram_parameter("sinx_t", [P, BT], F32, isOutput=False)
    consts_t = nc.declare_dram_parameter("consts_t", [1, P, P], BF16, isOutput=False)
    masks_t = nc.declare_dram_parameter("masks_t", [RB, P, QB], BF16, isOutput=False)
    y_c = nc.declare_dram_parameter("y_c", [BT, H], BF16, isOutput=True)

    scale = 1.0 / math.sqrt(P)

    with tile.TileContext(nc) as tc:
        with (
            tc.tile_pool(name="const", bufs=1) as constp,
            tc.tile_pool(name="persist", bufs=1) as persist,
            tc.tile_pool(name="qt", bufs=3 * NHC) as qt_p,
            tc.tile_pool(name="xin", bufs=HB + 2) as xin_p,
            tc.tile_pool(name="wqkv", bufs=1) as w_p,
            tc.tile_pool(name="tabs", bufs=2) as tab_p,
            tc.tile_pool(name="ropetmp", bufs=2) as rt_p,
            tc.tile_pool(name="pP", bufs=4) as p_p,
            tc.tile_pool(name="lacc", bufs=2) as lacc_p,
            tc.tile_pool(name="rt2", bufs=2) as rt2_pool,
            tc.tile_pool(name="aout", bufs=3 * NHC) as ao_p,
            tc.tile_pool(name="yout", bufs=2) as y_p,
            tc.tile_pool(name="psQS", bufs=2, space="PSUM") as qs_p,
            tc.tile_pool(name="psO", bufs=2, space="PSUM") as pso_p,
            tc.tile_pool(name="psV", bufs=2, space="PSUM") as psv_p,
            tc.tile_pool(name="psY", bufs=2, space="PSUM") as psy_p,
        ):
            # ones matrix for the L (softmax denominator) row-sum matmuls
            ones_sb = constp.tile([P, P], BF16, tag="ones")
            nc.sync.dma_start(out=ones_sb, in_=consts_t[0])
            mask_sb = [
                constp.tile([P, QB], BF16, tag=f"mask{o}", name=f"mask{o}")
                for o in range(RB)
            ]
            for o in range(RB):
                nc.sync.dma_start(out=mask_sb[o], in_=masks_t[o])

            # persistent per-core activations (bf16)
            kT = persist.tile([P, BT], BF16, tag="kT")
            vt = [
                persist.tile([P, P], BF16, tag=f"v{i}", name=f"v{i}")
                for i in range(BT // P)
            ]
            # o_proj weights (rows for this core's heads), resident whole run
            wo_sb = [
                persist.tile([P, H], BF16, tag=f"wo{h}", name=f"wo{h}")
                for h in range(NHC)
            ]
            # q/k/v projection weights, resident whole run
            wq_sb = [
                w_p.tile([P, DQ], BF16, tag=f"wq{i}", name=f"wq{i}")
                for i in range(HB)
            ]
            wk_sb = [
                w_p.tile([P, P], BF16, tag=f"wk{i}", name=f"wk{i}")
                for i in range(HB)
            ]
            wv_sb = [
                w_p.tile([P, P], BF16, tag=f"wv{i}", name=f"wv{i}")
                for i in range(HB)
            ]

            microq = []  # pending o_proj micro-ops

            def drain(n):
                for _ in range(min(n, len(microq))):
                    microq.pop(0)()

            def rope(dst, ps, cos_sb, sinx_sb):
                """dst = ps*cos + shift64(ps)*sinx (all [128,QB])"""
                t1 = rt_p.tile([P, QB], F32, tag="ropet1")
                t2 = rt_p.tile([P, QB], F32, tag="ropet2")
                nc.vector.tensor_tensor(t1, ps, cos_sb, mybir.AluOpType.mult)
                hh = P // 2
                nc.vector.tensor_tensor(
                    t2[0:hh], ps[hh:P], sinx_sb[0:hh], mybir.AluOpType.mult
                )
                nc.vector.tensor_tensor(
                    t2[hh:P], ps[0:hh], sinx_sb[hh:P], mybir.AluOpType.mult
                )
                nc.vector.tensor_tensor(dst, t1, t2, mybir.AluOpType.add)

            def emit_proj(tb):
                """Projections + rope for token block tb -> per-head q tiles."""
                t0 = tb * QB
                cos_sb = tab_p.tile([P, QB], F32, tag="cos")
                sinx_sb = tab_p.tile([P, QB], F32, tag="sinx")
                nc.sync.dma_start(out=cos_sb, in_=cos_t[:, t0 : t0 + QB])
                nc.sync.dma_start(out=sinx_sb, in_=sinx_t[:, t0 : t0 + QB])

                # stream xT tiles; on tb 0 interleave the weight loads so the
                # PE can start as soon as the first pairs arrive
                xts = []
                for hb in range(HB):
                    if tb == 0:
                        nc.sync.dma_start(
                            out=wq_sb[hb], in_=wq_c[hb * P : (hb + 1) * P, :]
                        )
                        nc.sync.dma_start(
                            out=wk_sb[hb], in_=wk_c[hb * P : (hb + 1) * P, :]
                        )
                        nc.sync.dma_start(
                            out=wv_sb[hb], in_=wv_c[hb * P : (hb + 1) * P, :]
                        )
                    xt = xin_p.tile([P, QB], BF16, tag="xin")
                    nc.sync.dma_start(
                        out=xt, in_=xT[hb * P : (hb + 1) * P, t0 : t0 + QB]
                    )
                    xts.append(xt)
                if tb == 0:
                    for h in range(NHC):
                        nc.sync.dma_start(
                            out=wo_sb[h], in_=wo_r[h * P : (h + 1) * P, :]
                        )

                qtiles = []
                for dq in range(NHC):
                    q_ps = qs_p.tile([P, QB], F32, tag="qs", name="qps")
                    for hb in range(HB):
                        nc.tensor.matmul(
                            q_ps,
                            wq_sb[hb][:, dq * P : (dq + 1) * P],
                            xts[hb],
                            start=(hb == 0),
                            stop=(hb == HB - 1),
                        )
                    qt = qt_p.tile([P, QB], BF16, tag="qt", name="qt")
                    rope(qt, q_ps, cos_sb, sinx_sb)
                    qtiles.append(qt)
                k_ps = qs_p.tile([P, QB], F32, tag="qs", name="kps")
                for hb in range(HB):
                    nc.tensor.matmul(
                        k_ps,
                        wk_sb[hb],
                        xts[hb],
                        start=(hb == 0),
                        stop=(hb == HB - 1),
                    )
                rope(kT[:, t0 : t0 + QB], k_ps, cos_sb, sinx_sb)
                for i in range(QB // P):
                    v_ps = psv_p.tile([P, P], F32, tag="vps")
                    for hb in range(HB):
                        nc.tensor.matmul(
                            v_ps,
                            xts[hb][:, i * P : (i + 1) * P],
                            wv_sb[hb],
                            start=(hb == 0),
                            stop=(hb == HB - 1),
                        )
                    nc.scalar.copy(vt[tb * (QB // P) + i], v_ps)
                return qtiles

            def emit_attn(b, qb, qtiles):
                """NHC heads of attention for q tokens [qb*QB,(qb+1)*QB) of
                batch b -> list of attn tiles [128 d, QB q] (bf16)."""
                attn_tiles = []
                nkb = (qb + 1) * RB
                for h in range(NHC):
                    o_ps = pso_p.tile([P, QB], F32, tag="ops")
                    la0 = lacc_p.tile([P, QB], BF16, tag="la0", name="la0")
                    la1 = lacc_p.tile([P, QB], BF16, tag="la1", name="la1")
                    la = [la0, la1]
                    for kb in range(nkb):
                        k0 = b * S + kb * KB
                        s_ps = qs_p.tile([P, QB], F32, tag="qs", name="sps")
                        nc.tensor.matmul(
                            s_ps,
                            kT[:, k0 : k0 + KB],
                            qtiles[h],
                            start=True,
                            stop=True,
                        )
                        p_sb = p_p.tile([P, QB], BF16, tag="P")
                        nc.scalar.activation(
                            p_sb,
                            s_ps,
                            mybir.ActivationFunctionType.Exp,
                            scale=scale,
                        )
                        o = kb - qb * RB
                        if o >= 0:
                            nc.vector.tensor_tensor(
                                p_sb, p_sb, mask_sb[o], mybir.AluOpType.mult
                            )
                        nc.tensor.matmul(
                            o_ps,
                            vt[k0 // P],
                            p_sb,
                            start=(kb == 0),
                            stop=(kb == nkb - 1),
                        )
                        # softmax denominator: two bf16 accumulators on DVE
                        if kb < 2:
                            nc.vector.tensor_copy(la[kb], p_sb)
                        else:
                            nc.vector.tensor_tensor(
                                la[kb & 1], la[kb & 1], p_sb, mybir.AluOpType.add
                            )
                        drain(1)
                    # partition-sum broadcast of L via two tiny PE matmuls
                    l_ps = qs_p.tile([P, QB], F32, tag="qs", name="lps")
                    nc.tensor.matmul(l_ps, ones_sb, la0, start=True, stop=False)
                    nc.tensor.matmul(l_ps, ones_sb, la1, start=False, stop=True)
                    rinv = rt2_pool.tile([P, QB], F32, tag="rinv")
                    nc.vector.reciprocal_approx_fast(out=rinv, in_=l_ps)
                    attn_sb = ao_p.tile([P, QB], BF16, tag="attn")
                    nc.vector.tensor_tensor(
                        attn_sb, o_ps, rinv, mybir.AluOpType.mult
                    )
                    attn_tiles.append(attn_sb)
                    drain(1)
                return attn_tiles

            def push_oproj(b, qb, attn_tiles):
                """Queue partial o_proj micro-ops for chunk (b, qb):
                y[t0+ti*P, :] = sum_h attn_h[:, ti]^T @ wo_rows_h."""
                t0 = b * S + qb * QB
                for ti in range(QB // P):
                    holder = {}
                    for nch in range(H // QB):

                        def op(ti=ti, nch=nch, holder=holder, tiles=attn_tiles, t0=t0):
                            if nch == 0:
                                holder["y"] = y_p.tile(
                                    [P, H], BF16, tag="ysb", name="ysb"
                                )
                            y_sb = holder["y"]
                            y_ps = psy_p.tile([P, QB], F32, tag="yps")
                            for h in range(NHC):
                                nc.tensor.matmul(
                                    y_ps,
                                    tiles[h][:, ti * P : (ti + 1) * P],
                                    wo_sb[h][:, nch * QB : (nch + 1) * QB],
                                    start=(h == 0),
                                    stop=(h == NHC - 1),
                                )
                            dst = y_sb[:, nch * QB : (nch + 1) * QB]
                            if nch % 2 == 0:
                                nc.scalar.copy(dst, y_ps)
                            else:
                                nc.vector.tensor_copy(dst, y_ps)
                            if nch == H // QB - 1:
                                nc.sync.dma_start(
                                    out=y_c[t0 + ti * P : t0 + (ti + 1) * P, :],
                                    in_=y_sb,
                                )

                        microq.append(op)

            chunks = [(b, qb) for b in range(B) for qb in range(QBB)]
            for i, (b, qb) in enumerate(chunks):
                qtiles = emit_proj(i)
                atiles = emit_attn(b, qb, qtiles)
                push_oproj(b, qb, atiles)
            drain(len(microq))
    nc.finalize()
    return nc


def _prep_inputs(hidden_states, wq, wk, wv, wo, position_ids, B, S, H, NH, NKV):
    """Host-side: bf16 casts, x transpose, rope tables, masks, per-core slices."""
    BT = B * S
    NHC = NH // NCORES
    DQ = NHC * P
    RB = QB // KB

    bf = ml_dtypes.bfloat16
    xT = np.ascontiguousarray(
        np.asarray(hidden_states).reshape(BT, H).T
    ).astype(bf)
    wq_b, wk_b, wv_b, wo_b = (np.asarray(w).astype(bf) for w in (wq, wk, wv, wo))

    # rope tables in transposed layout [128 d, BT t]
    half = P // 2
    inv_freq = 1.0 / (10000.0 ** (np.arange(half, dtype=np.float64) / half))
    pos = np.asarray(position_ids).astype(np.float64).reshape(BT)
    ang = pos[None, :] * inv_freq[:, None]  # [64, BT]
    cos_t = np.concatenate([np.cos(ang), np.cos(ang)], 0).astype(np.float32)
    sinx_t = np.concatenate([-np.sin(ang), np.sin(ang)], 0).astype(np.float32)

    consts_t = np.ones((1, P, P)).astype(bf)
    # full-width diagonal-block masks: mask[o][k, q] = 1 if o*KB + k <= q
    kk = np.arange(KB)[None, :, None]
    qq = np.arange(QB)[None, None, :]
    oo = np.arange(RB)[:, None, None]
    masks_t = ((oo * KB + kk) <= qq).astype(bf)

    in_maps = []
    for c in range(NCORES):
        in_maps.append(
            {
                "xT": xT,
                "wq_c": np.ascontiguousarray(wq_b[:, c * DQ : (c + 1) * DQ]),
                "wk_c": np.ascontiguousarray(wk_b[:, c * P : (c + 1) * P]),
                "wv_c": np.ascontiguousarray(wv_b[:, c * P : (c + 1) * P]),
                "wo_r": np.ascontiguousarray(wo_b[c * DQ : (c + 1) * DQ, :]),
                "cos_t": cos_t,
                "sinx_t": sinx_t,
                "consts_t": consts_t,
                "masks_t": masks_t,
            }
        )
    return in_maps


def run(hidden_states, wq, wk, wv, wo, position_ids, B, S, H, NH, NKV, trace=False):
    key = (B, S, H, NH, NKV)
    if key not in _CACHE:
        _CACHE[key] = build_program(B, S, H, NH, NKV)
    nc = _CACHE[key]
    in_maps = _prep_inputs(
        hidden_states, wq, wk, wv, wo, position_ids, B, S, H, NH, NKV
    )
    res = run_bass_kernel_spmd(nc, in_maps, core_ids=list(range(NCORES)), trace=trace)
    acc = np.zeros((B * S, H), np.float32)
    for c in range(NCORES):
        acc += np.asarray(res.results[c]["y_c"], dtype=np.float32)
    out = acc.reshape(B, S, H)
    return (out, res) if trace else (out, None)


def kernel(hidden_states, wq, wk, wv, wo, position_ids):
    out, _ = run(
        hidden_states, wq, wk, wv, wo, position_ids, 2, 2048, 4096, 32, 8
    )
    return out


# revision 31
# speedup vs baseline: 1.1332x; 1.0142x over previous
"""Tensor-parallel LlamaAttention (GQA + RoPE + causal) for 8 trn2 NeuronCores.

Sharding: column-parallel q/k/v by head (4 q-heads, 1 kv-head per core),
attention computed locally per head, then ROW-parallel o_proj on the local
head slice producing a partial output y_part[BT, H]; the host sums the 8
partials (the "all-reduce" of the RowParallel structure is done at gather
time on the host, so no on-device collective is needed).

Layout strategy (token dim = free dim, everything transposed):
  xT[h, t]     provided by the host (pre-transposed, bf16)
  qT/kT[d, t]  from projection matmuls (lhsT=W block, rhs=xT block)
  v[t, d]      natural layout (lhsT=xT block, rhs=Wv block)
  S^T[k, q]    = matmul(lhsT=kT slice, rhs=qT slice)
  P^T          = exp(scale*S^T) via ACT (scores bounded ~|8|, exp safe)
  O^T[d, q]   += matmul(lhsT=v tile, rhs=P^T)    (PSUM accum over k)
  L (denoms)   = two bf16 DVE accumulators of P tiles, then a tiny
                 ones-matmul on PE broadcasts the partition-sum into PSUM
  attnT        = O^T * reciprocal_approx_fast(L) (SBUF, bf16)
  y_part      += matmul(lhsT=attnT slice, rhs=wo rows) -> DRAM, host-summed

Pipelining: projection chunk i (512 tokens), attention chunk i, and o_proj
micro-ops for chunk i-1 are interleaved in the instruction stream, so the
PE always has independent work during DMA warmup and exp-latency stalls.
"""

import math
import sys

import numpy as np

sys.path.insert(0, "/opt/trn_rl_repo")

import ml_dtypes  # noqa: E402

from concourse import bacc, mybir, tile  # noqa: E402
from concourse.bass_utils import run_bass_kernel_spmd  # noqa: E402

F32 = mybir.dt.float32
BF16 = mybir.dt.bfloat16
NCORES = 8
P = 128  # partitions / head dim
QB = 512  # q-block (PSUM free dim)
KB = 128  # k-block (contraction tile)

_CACHE = {}


def build_program(B, S, H, NH, NKV):
    nc = bacc.Bacc("TRN2", num_devices=NCORES)

    BT = B * S  # total tokens (4096)
    NHC = NH // NCORES  # q heads per core (4)
    DQ = NHC * P  # per-core q width (512)
    HB = H // P  # h blocks (32)
    TB = BT // QB  # token super-blocks (8)
    QBB = S // QB  # q blocks per batch (4)
    RB = QB // KB  # diag tiles per q block (4)

    xT = nc.declare_dram_parameter("xT", [H, BT], BF16, isOutput=False)
    wq_c = nc.declare_dram_parameter("wq_c", [H, DQ], BF16, isOutput=False)
    wk_c = nc.declare_dram_parameter("wk_c", [H, P], BF16, isOutput=False)
    wv_c = nc.declare_dram_parameter("wv_c", [H, P], BF16, isOutput=False)
    wo_r = nc.declare_dram_parameter("wo_r", [DQ, H], BF16, isOutput=False)
    cos_t = nc.declare_dram_parameter("cos_t", [P, BT], F32, isOutput=False)
    sinx_t = nc.declare_dram_parameter("sinx_t", [P, BT], F32, isOutput=False)
    consts_t = nc.declare_dram_parameter("consts_t", [2, P, P], BF16, isOutput=False)
    y_c = nc.declare_dram_parameter("y_c", [BT, H], BF16, isOutput=True)

    scale = 1.0 / math.sqrt(P)

    with tile.TileContext(nc) as tc:
        with (
            tc.tile_pool(name="const", bufs=1) as constp,
            tc.tile_pool(name="persist", bufs=1) as persist,
            tc.tile_pool(name="qt", bufs=3 * NHC) as qt_p,
            tc.tile_pool(name="xin", bufs=HB + 2) as xin_p,
            tc.tile_pool(name="wqkv", bufs=1) as w_p,
            tc.tile_pool(name="tabs", bufs=2) as tab_p,
            tc.tile_pool(name="ropetmp", bufs=2) as rt_p,
            tc.tile_pool(name="pP", bufs=4) as p_p,
            tc.tile_pool(name="lacc", bufs=2) as lacc_p,
            tc.tile_pool(name="rt2", bufs=2) as rt2_pool,
            tc.tile_pool(name="aout", bufs=3 * NHC) as ao_p,
            tc.tile_pool(name="yout", bufs=2) as y_p,
            tc.tile_pool(name="psQS", bufs=2, space="PSUM") as qs_p,
            tc.tile_pool(name="psO", bufs=2, space="PSUM") as pso_p,
            tc.tile_pool(name="psV", bufs=2, space="PSUM") as psv_p,
            tc.tile_pool(name="psY", bufs=2, space="PSUM") as psy_p,
        ):
            # ones matrix for the L (softmax denominator) row-sum matmuls
            ones_sb = constp.tile([P, P], BF16, tag="ones")
            # triangular causal mask for the diagonal subtile: tri[k,q]=k<=q
            tri_sb = constp.tile([P, P], BF16, tag="tri")

            # persistent per-core activations (bf16)
            kT = persist.tile([P, BT], BF16, tag="kT")
            vt = [
                persist.tile([P, P], BF16, tag=f"v{i}", name=f"v{i}")
                for i in range(BT // P)
            ]
            # o_proj weights (rows for this core's heads), resident whole run
            wo_sb = [
                persist.tile([P, H], BF16, tag=f"wo{h}", name=f"wo{h}")
                for h in range(NHC)
            ]
            # q/k/v projection weights, resident whole run
            wq_sb = [
                w_p.tile([P, DQ], BF16, tag=f"wq{i}", name=f"wq{i}")
                for i in range(HB)
            ]
            wk_sb = [
                w_p.tile([P, P], BF16, tag=f"wk{i}", name=f"wk{i}")
                for i in range(HB)
            ]
            wv_sb = [
                w_p.tile([P, P], BF16, tag=f"wv{i}", name=f"wv{i}")
                for i in range(HB)
            ]

            microq = []  # pending o_proj micro-ops

            def drain(n):
                for _ in range(min(n, len(microq))):
                    microq.pop(0)()

            def rope(dst, ps, cos_sb, sinx_sb):
                """dst = ps*cos + shift64(ps)*sinx (all [128,QB])"""
                t1 = rt_p.tile([P, QB], F32, tag="ropet1")
                t2 = rt_p.tile([P, QB], F32, tag="ropet2")
                nc.vector.tensor_tensor(t1, ps, cos_sb, mybir.AluOpType.mult)
                hh = P // 2
                nc.vector.tensor_tensor(
                    t2[0:hh], ps[hh:P], sinx_sb[0:hh], mybir.AluOpType.mult
                )
                nc.vector.tensor_tensor(
                    t2[hh:P], ps[0:hh], sinx_sb[hh:P], mybir.AluOpType.mult
                )
                nc.vector.tensor_tensor(dst, t1, t2, mybir.AluOpType.add)

            def emit_proj(tb):
                """Projections + rope for token block tb -> per-head q tiles."""
                t0 = tb * QB
                cos_sb = tab_p.tile([P, QB], F32, tag="cos")
                sinx_sb = tab_p.tile([P, QB], F32, tag="sinx")
                nc.sync.dma_start(out=cos_sb, in_=cos_t[:, t0 : t0 + QB])
                nc.sync.dma_start(out=sinx_sb, in_=sinx_t[:, t0 : t0 + QB])

                # stream xT tiles; on tb 0 interleave the weight loads so the
                # PE can start as soon as the first pairs arrive
                xts = []
                for hb in range(HB):
                    if tb == 0:
                        nc.sync.dma_start(
                            out=wq_sb[hb], in_=wq_c[hb * P : (hb + 1) * P, :]
                        )
                        nc.sync.dma_start(
                            out=wk_sb[hb], in_=wk_c[hb * P : (hb + 1) * P, :]
                        )
                        nc.sync.dma_start(
                            out=wv_sb[hb], in_=wv_c[hb * P : (hb + 1) * P, :]
                        )
                    xt = xin_p.tile([P, QB], BF16, tag="xin")
                    nc.sync.dma_start(
                        out=xt, in_=xT[hb * P : (hb + 1) * P, t0 : t0 + QB]
                    )
                    xts.append(xt)
                if tb == 0:
                    # consts are first needed by chunk-0 attention; keep them
                    # out of the warmup-critical DMA window
                    nc.sync.dma_start(out=ones_sb, in_=consts_t[0])
                    nc.sync.dma_start(out=tri_sb, in_=consts_t[1])
                if tb == 1:
                    # o_proj weights are first needed by the chunk-0 drains
                    for h in range(NHC):
                        nc.sync.dma_start(
                            out=wo_sb[h], in_=wo_r[h * P : (h + 1) * P, :]
                        )

                qtiles = []
                for dq in range(NHC):
                    q_ps = qs_p.tile([P, QB], F32, tag="qs", name="qps")
                    for hb in range(HB):
                        nc.tensor.matmul(
                            q_ps,
                            wq_sb[hb][:, dq * P : (dq + 1) * P],
                            xts[hb],
                            start=(hb == 0),
                            stop=(hb == HB - 1),
                        )
                    qt = qt_p.tile([P, QB], BF16, tag="qt", name="qt")
                    rope(qt, q_ps, cos_sb, sinx_sb)
                    qtiles.append(qt)
                k_ps = qs_p.tile([P, QB], F32, tag="qs", name="kps")
                for hb in range(HB):
                    nc.tensor.matmul(
                        k_ps,
                        wk_sb[hb],
                        xts[hb],
                        start=(hb == 0),
                        stop=(hb == HB - 1),
                    )
                rope(kT[:, t0 : t0 + QB], k_ps, cos_sb, sinx_sb)
                for i in range(QB // P):
                    v_ps = psv_p.tile([P, P], F32, tag="vps")
                    for hb in range(HB):
                        nc.tensor.matmul(
                            v_ps,
                            xts[hb][:, i * P : (i + 1) * P],
                            wv_sb[hb],
                            start=(hb == 0),
                            stop=(hb == HB - 1),
                        )
                    nc.scalar.copy(vt[tb * (QB // P) + i], v_ps)
                return qtiles

            def emit_attn(b, qb, qtiles):
                """NHC heads of attention for q tokens [qb*QB,(qb+1)*QB) of
                batch b -> list of attn tiles [128 d, QB q] (bf16)."""
                attn_tiles = []
                nkb = (qb + 1) * RB
                for h in range(NHC):
                    o_ps = pso_p.tile([P, QB], F32, tag="ops")
                    la0 = lacc_p.tile([P, QB], BF16, tag="la0", name="la0")
                    la1 = lacc_p.tile([P, QB], BF16, tag="la1", name="la1")
                    la = [la0, la1]
                    for kb in range(nkb):
                        k0 = b * S + kb * KB
                        o = kb - qb * RB  # >=0 on the diagonal supertile
                        c0 = max(o, 0) * KB  # first q column attending to kb
                        s_ps = qs_p.tile([P, QB], F32, tag="qs", name="sps")
                        nc.tensor.matmul(
                            s_ps[:, c0:QB],
                            kT[:, k0 : k0 + KB],
                            qtiles[h][:, c0:QB],
                            start=True,
                            stop=True,
                        )
                        p_sb = p_p.tile([P, QB], BF16, tag="P")
                        if o > 0:
                            # columns below the diagonal never attend to kb
                            nc.gpsimd.memset(p_sb[:, 0:c0], 0.0)
                        nc.scalar.activation(
                            p_sb[:, c0:QB],
                            s_ps[:, c0:QB],
                            mybir.ActivationFunctionType.Exp,
                            scale=scale,
                        )
                        if o >= 0:
                            nc.vector.tensor_tensor(
                                p_sb[:, c0 : c0 + KB],
                                p_sb[:, c0 : c0 + KB],
                                tri_sb,
                                mybir.AluOpType.mult,
                            )
                        nc.tensor.matmul(
                            o_ps,
                            vt[k0 // P],
                            p_sb,
                            start=(kb == 0),
                            stop=(kb == nkb - 1),
                        )
                        # softmax denominator: two bf16 accumulators on DVE
                        if kb < 2:
                            nc.vector.tensor_copy(la[kb], p_sb)
                        else:
                            nc.vector.tensor_tensor(
                                la[kb & 1], la[kb & 1], p_sb, mybir.AluOpType.add
                            )
                        drain(1)
                    # partition-sum broadcast of L via two tiny PE matmuls
                    l_ps = qs_p.tile([P, QB], F32, tag="qs", name="lps")
                    nc.tensor.matmul(l_ps, ones_sb, la0, start=True, stop=False)
                    nc.tensor.matmul(l_ps, ones_sb, la1, start=False, stop=True)
                    rinv = rt2_pool.tile([P, QB], F32, tag="rinv")
                    nc.vector.reciprocal_approx_fast(out=rinv, in_=l_ps)
                    attn_sb = ao_p.tile([P, QB], BF16, tag="attn")
                    nc.vector.tensor_tensor(
                        attn_sb, o_ps, rinv, mybir.AluOpType.mult
                    )
                    attn_tiles.append(attn_sb)
                    drain(1)
                return attn_tiles

            def push_oproj(b, qb, attn_tiles):
                """Queue partial o_proj micro-ops for chunk (b, qb):
                y[t0+ti*P, :] = sum_h attn_h[:, ti]^T @ wo_rows_h."""
                t0 = b * S + qb * QB
                for ti in range(QB // P):
                    holder = {}
                    for nch in range(H // QB):

                        def op(ti=ti, nch=nch, holder=holder, tiles=attn_tiles, t0=t0):
                            if nch == 0:
                                holder["y"] = y_p.tile(
                                    [P, H], BF16, tag="ysb", name="ysb"
                                )
                            y_sb = holder["y"]
                            y_ps = psy_p.tile([P, QB], F32, tag="yps")
                            for h in range(NHC):
                                nc.tensor.matmul(
                                    y_ps,
                                    tiles[h][:, ti * P : (ti + 1) * P],
                                    wo_sb[h][:, nch * QB : (nch + 1) * QB],
                                    start=(h == 0),
                                    stop=(h == NHC - 1),
                                )
                            dst = y_sb[:, nch * QB : (nch + 1) * QB]
                            if nch % 2 == 0:
                                nc.scalar.copy(dst, y_ps)
                            else:
                                nc.vector.tensor_copy(dst, y_ps)
                            if nch == H // QB - 1:
                                nc.sync.dma_start(
                                    out=y_c[t0 + ti * P : t0 + (ti + 1) * P, :],
                                    in_=y_sb,
                                )

                        microq.append(op)

            chunks = [(b, qb) for b in range(B) for qb in range(QBB)]
            for i, (b, qb) in enumerate(chunks):
                qtiles = emit_proj(i)
                atiles = emit_attn(b, qb, qtiles)
                push_oproj(b, qb, atiles)
            drain(len(microq))
    nc.finalize()
    return nc


def _prep_inputs(hidden_states, wq, wk, wv, wo, position_ids, B, S, H, NH, NKV):
    """Host-side: bf16 casts, x transpose, rope tables, masks, per-core slices."""
    BT = B * S
    NHC = NH // NCORES
    DQ = NHC * P
    RB = QB // KB

    bf = ml_dtypes.bfloat16
    xT = np.ascontiguousarray(
        np.asarray(hidden_states).reshape(BT, H).T
    ).astype(bf)
    wq_b, wk_b, wv_b, wo_b = (np.asarray(w).astype(bf) for w in (wq, wk, wv, wo))

    # rope tables in transposed layout [128 d, BT t]
    half = P // 2
    inv_freq = 1.0 / (10000.0 ** (np.arange(half, dtype=np.float64) / half))
    pos = np.asarray(position_ids).astype(np.float64).reshape(BT)
    ang = pos[None, :] * inv_freq[:, None]  # [64, BT]
    cos_t = np.concatenate([np.cos(ang), np.cos(ang)], 0).astype(np.float32)
    sinx_t = np.concatenate([-np.sin(ang), np.sin(ang)], 0).astype(np.float32)

    # consts: [ones for L row-sums, triangular causal mask tri[k,q]=k<=q]
    tri = np.arange(P)[:, None] <= np.arange(P)[None, :]
    consts_t = np.stack([np.ones((P, P)), tri]).astype(bf)

    in_maps = []
    for c in range(NCORES):
        in_maps.append(
            {
                "xT": xT,
                "wq_c": np.ascontiguousarray(wq_b[:, c * DQ : (c + 1) * DQ]),
                "wk_c": np.ascontiguousarray(wk_b[:, c * P : (c + 1) * P]),
                "wv_c": np.ascontiguousarray(wv_b[:, c * P : (c + 1) * P]),
                "wo_r": np.ascontiguousarray(wo_b[c * DQ : (c + 1) * DQ, :]),
                "cos_t": cos_t,
                "sinx_t": sinx_t,
                "consts_t": consts_t,
            }
        )
    return in_maps


def run(hidden_states, wq, wk, wv, wo, position_ids, B, S, H, NH, NKV, trace=False):
    key = (B, S, H, NH, NKV)
    if key not in _CACHE:
        _CACHE[key] = build_program(B, S, H, NH, NKV)
    nc = _CACHE[key]
    in_maps = _prep_inputs(
        hidden_states, wq, wk, wv, wo, position_ids, B, S, H, NH, NKV
    )
    res = run_bass_kernel_spmd(nc, in_maps, core_ids=list(range(NCORES)), trace=trace)
    acc = np.zeros((B * S, H), np.float32)
    for c in range(NCORES):
        acc += np.asarray(res.results[c]["y_c"], dtype=np.float32)
    out = acc.reshape(B, S, H)
    return (out, res) if trace else (out, None)


def kernel(hidden_states, wq, wk, wv, wo, position_ids):
    out, _ = run(
        hidden_states, wq, wk, wv, wo, position_ids, 2, 2048, 4096, 32, 8
    )
    return out
